# revision 13
# baseline (speedup 1.0000x reference)
"""KnowledgeRNN Trainium2 kernel: 8-core SPMD.

Device (Bass/Tile, 8 NeuronCores), bf16 matmuls (fp32 PSUM accumulate):
  - Phase A: batched input projections  XP = X @ [Wq1_x | W_ih_x^T]
    (output-dim sharded 8 ways, 768 cols/core), fp32 output.
  - Phase B: decoder  logits = F @ W_dec^T  (vocab sharded 8 ways,
    4000 cols/core), bf16 logits output.
  X (the shared activation matrix) is loaded into SBUF once per core and
  kept resident; W streams through double-buffered tiles; matmuls use the
  full 1024-wide bf16 moving operand (2 PSUM banks per tile).
Host: embedding gather, bias adds, the 2048-step sequential scan glue
(state-dependent matvecs), final log_softmax normalization.
"""
import os
import sys
import time

sys.path.insert(0, '/opt/trn_rl_repo')
sys.path.insert(0, '/opt/trn_rl_repo/concourse')
os.environ.setdefault("MYCRO_LOCAL_CACHE", "1")

import numpy as np
import ml_dtypes

import concourse.bass as bass
import concourse.mybir as mybir
from concourse import bacc, tile, bass_utils
from concourse.tile import add_dep_helper

N_CORES = 8
NTOK, STATE, EMB = 32000, 1024, 1024
QUERY, VALUE, NKB = 256, 512, 10000
SEQ = 2048
QIN = STATE + EMB
DEC_IN = STATE + EMB + VALUE

F32 = mybir.dt.float32
BF16 = mybir.dt.bfloat16
NP_BF16 = ml_dtypes.bfloat16

# enable NTFF tracing for local perf runs (test.py sets this)
_TRACE = os.environ.get("KRNN_TRACE", "") == "1"


def _build_mm_kernel(K, S, N, out_dt=BF16, NW=512):
    """OUT[S,N] = XT^T @ W  (no bias — host adds it).

    Inputs (per core): "xt" [K,S] bf16, "w" [K,N] bf16.
    Output: "out" [S,N] out_dt.
    """
    assert K % 128 == 0 and S % 128 == 0
    KC = K // 128
    ST = S // 128
    nbs = []
    o = 0
    while o < N:
        w = min(NW, N - o)
        nbs.append((o, w))
        o += w
    # Narrowest block first: the first W block's DMA gates the first
    # matmul, so a smaller head block starts the PE earlier.
    nbs.sort(key=lambda t: t[1])

    nc = bacc.Bacc(None, target_bir_lowering=False)
    xt = nc.declare_dram_parameter("xt", [K, S], BF16, isOutput=False)
    wt = nc.declare_dram_parameter("w", [K, N], BF16, isOutput=False)
    out = nc.declare_dram_parameter("out", [S, N], out_dt, isOutput=True)

    xt_v = xt.rearrange("(kb p) s -> p kb s", p=128)
    wt_v = wt.rearrange("(kb p) n -> p kb n", p=128)

    # X resident in SBUF, loaded once as a chained sequence of chunks along
    # S: a small head chunk so the first matmul starts early, then 512-col
    # chunks each DMA-gated on the previous one so they never steal HBM
    # bandwidth from the transfer the PE is actually waiting on.
    if S >= 1024:
        xcols = [256] + [512] * ((S - 512) // 512) + [256]
    else:
        xcols = [S]
    xoffs = [sum(xcols[:i]) for i in range(len(xcols))]

    with tile.TileContext(nc) as tc:
        with (
            tc.tile_pool(name="xres", bufs=1) as xres,
            tc.tile_pool(name="wpool", bufs=2) as wpool,
            tc.tile_pool(name="opool", bufs=3) as opool,
            tc.tile_pool(name="ppool", bufs=3, space="PSUM") as ppool,
        ):
            # Startup bandwidth management: issue W block 0 first, then the
            # small X head chunk; only those two compete for HBM before the
            # first matmul. Later X chunks chain on the previous chunk's
            # DMA; the W block 1 prefetch waits for the mid X chunk.
            wblk0 = wpool.tile([128, KC, NW], BF16, tag="w", name="wblk0")
            nbo0, nbw0 = nbs[0]
            nc.sync.dma_start(out=wblk0[:, :, :nbw0], in_=wt_v[:, :, nbo0:nbo0 + nbw0])

            xch = []
            xdmas = []
            for c, (xo_c, xw_c) in enumerate(zip(xoffs, xcols)):
                xc_t = xres.tile([128, KC, xw_c], BF16, tag=f"x{c}", name=f"x{c}")
                xd = nc.sync.dma_start(
                    out=xc_t[:, :, :], in_=xt_v[:, :, xo_c:xo_c + xw_c]
                )
                if c > 0:
                    add_dep_helper(
                        xd.ins, xdmas[c - 1].ins, sync=True,
                        reason="chain x chunk loads",
                    )
                xch.append(xc_t)
                xdmas.append(xd)

            def x_slice(st):
                pos = st * 128
                for c in range(len(xcols)):
                    if pos < xoffs[c] + xcols[c]:
                        return xch[c], pos - xoffs[c]
                raise AssertionError

            w1_gate = xdmas[min(2, len(xdmas) - 1)]
            for nbi, (nbo, nbw) in enumerate(nbs):
                if nbi == 0:
                    wblk = wblk0
                else:
                    wblk = wpool.tile([128, KC, NW], BF16, tag="w", name=f"wblk{nbi}")
                    wd = nc.sync.dma_start(
                        out=wblk[:, :, :nbw], in_=wt_v[:, :, nbo:nbo + nbw]
                    )
                    if nbi == 1:
                        add_dep_helper(
                            wd.ins, w1_gate.ins, sync=True,
                            reason="hold W prefetch off the startup stream",
                        )
                for st in range(ST):
                    xc, xo = x_slice(st)
                    ps = ppool.tile([128, NW], F32, tag="ps")
                    for kb in range(KC):
                        nc.tensor.matmul(
                            ps[:, :nbw], xc[:, kb, xo:xo + 128], wblk[:, kb, :nbw],
                            start=(kb == 0), stop=(kb == KC - 1),
                        )
                    ot = opool.tile([128, NW], out_dt, tag="o")
                    nc.vector.tensor_copy(out=ot[:, :nbw], in_=ps[:, :nbw])
                    nc.sync.dma_start(
                        out=out[st * 128:(st + 1) * 128, nbo:nbo + nbw],
                        in_=ot[:, :nbw],
                    )
    nc.compile()
    return nc


_KERNEL_CACHE = {}
LAST_EXEC_NS = 0


def _run_mm(key, K, S, N, xts, ws, out_dt=BF16, NW=512):
    global LAST_EXEC_NS
    if key not in _KERNEL_CACHE:
        _KERNEL_CACHE[key] = _build_mm_kernel(K, S, N, out_dt, NW)
    nc = _KERNEL_CACHE[key]
    in_maps = [
        {"xt": np.ascontiguousarray(xts[c]),
         "w": np.ascontiguousarray(ws[c])}
        for c in range(N_CORES)
    ]
    kwargs = {}
    if _TRACE:
        kwargs["trace"] = True
    res = bass_utils.run_bass_kernel_spmd(
        nc, in_maps, core_ids=list(range(N_CORES)), **kwargs
    )
    if res.exec_time_ns:
        LAST_EXEC_NS += res.exec_time_ns
    return res


def kernel(input_ids, enc_W, Wq1, bq1, Wq2, bq2, kb_keys, kb_vals,
           W_ih, b_ih, W_hh, b_hh, W_dec, b_dec):
    input_ids = np.asarray(input_ids)
    enc_W = np.asarray(enc_W, np.float32)
    Wq1 = np.asarray(Wq1, np.float32)
    bq1 = np.asarray(bq1, np.float32)
    Wq2 = np.asarray(Wq2, np.float32)
    bq2 = np.asarray(bq2, np.float32)
    kb_keys = np.asarray(kb_keys, np.float32)
    kb_vals = np.asarray(kb_vals, np.float32)
    W_ih = np.asarray(W_ih, np.float32)
    b_ih = np.asarray(b_ih, np.float32)
    W_hh = np.asarray(W_hh, np.float32)
    b_hh = np.asarray(b_hh, np.float32)
    W_dec = np.asarray(W_dec, np.float32)
    b_dec = np.asarray(b_dec, np.float32)

    # ---- embedding gather (host glue) ----
    emb = enc_W[input_ids]                      # [S, EMB] fp32
    X_T16 = np.ascontiguousarray(emb.T.astype(NP_BF16))   # [EMB, S] bf16

    # ---- Phase A on device: XP = X @ [Wq1_x | W_ih_x^T]; bias added on host
    Wq1_x = Wq1[STATE:, :]                      # [1024, 2048]
    W_ih_xT = np.ascontiguousarray(W_ih[:, :EMB].T)   # [1024, 4096]
    PROJ = np.concatenate([Wq1_x, W_ih_xT], axis=1).astype(NP_BF16)  # [1024, 6144]
    BIAS = np.concatenate([bq1, b_ih + b_hh]).astype(np.float32)     # [6144]
    NSH = 6144 // N_CORES                              # 768
    ws = [PROJ[:, c * NSH:(c + 1) * NSH] for c in range(N_CORES)]
    xts = [X_T16] * N_CORES
    resA = _run_mm("A", EMB, SEQ, NSH, xts, ws, out_dt=F32)
    XP = np.concatenate([resA.results[c]["out"] for c in range(N_CORES)], axis=1)
    XP += BIAS[None, :]
    xq_pre = XP[:, :2048]                        # [S, 2048]  (= x@Wq1_x + bq1)
    xg_pre = XP[:, 2048:]                        # [S, 4096]  (= x@W_ih_x^T + b_ih + b_hh)

    # ---- host sequential scan (glue around device-precomputed projections) ----
    Wq1_h = np.ascontiguousarray(Wq1[:STATE, :])       # [1024, 2048]
    HXW = np.concatenate([Wq1_h, W_hh.T], axis=1)      # [1024, 2048+4096]
    HXW = np.ascontiguousarray(HXW)
    W_ihvT = np.ascontiguousarray(W_ih[:, EMB:].T)     # [512, 4096]
    kb_keys_c = np.ascontiguousarray(kb_keys)
    kb_vals_c = np.ascontiguousarray(kb_vals)
    Wq2_c = np.ascontiguousarray(Wq2)

    hx = np.zeros(STATE, np.float32)
    cx = np.zeros(STATE, np.float32)
    lstm_states = np.empty((SEQ, STATE), np.float32)
    kb_out = np.empty((SEQ, VALUE), np.float32)
    _t0 = time.time()
    for t in range(SEQ):
        if t % 512 == 0:
            print(f"[kernel] scan step {t} ({time.time()-_t0:.1f}s)", flush=True)
        lstm_states[t] = hx
        hp = hx @ HXW                                  # [6144]
        qh = np.tanh(hp[:2048] + xq_pre[t])
        q = qh @ Wq2_c + bq2                           # [256]
        sc = kb_keys_c @ q                             # [NKB]
        sc -= sc.max()
        u = np.exp(sc)
        attn = u / u.sum()
        val = attn @ kb_vals_c                         # [512]
        kb_out[t] = val
        gates = xg_pre[t] + val @ W_ihvT + hp[2048:]   # [4096]
        i_g = gates[:1024]
        f_g = gates[1024:2048]
        g_g = gates[2048:3072]
        o_g = gates[3072:]
        sig_i = 1.0 / (1.0 + np.exp(-i_g))
        sig_f = 1.0 / (1.0 + np.exp(-f_g))
        sig_o = 1.0 / (1.0 + np.exp(-o_g))
        cx = sig_f * cx + sig_i * np.tanh(g_g)
        hx = sig_o * np.tanh(cx)

    # ---- Phase B on device: decoder (host adds bias + log_softmax) ----
    F = np.concatenate([emb, kb_out, lstm_states], axis=1)   # [S, 2560]
    F_T16 = np.ascontiguousarray(F.T.astype(NP_BF16))        # [2560, S] bf16
    VSH = NTOK // N_CORES                                    # 4000
    wdt16 = np.ascontiguousarray(W_dec.T.astype(NP_BF16))    # [2560, 32000]
    ws_b = [np.ascontiguousarray(wdt16[:, c * VSH:(c + 1) * VSH]) for c in range(N_CORES)]
    xts_b = [F_T16] * N_CORES
    resB = _run_mm("B", DEC_IN, SEQ, VSH, xts_b, ws_b, out_dt=BF16)

    logits = np.concatenate(
        [resB.results[c]["out"].astype(np.float32) for c in range(N_CORES)], axis=1
    )
    if os.environ.get("KRNN_DUMP_CACHE"):
        np.savez("/tmp/krnn_f_cache.npz",
                 F_T16=F_T16.view(np.uint16), wdt16=wdt16.view(np.uint16),
                 logits=(F @ W_dec.T.astype(np.float32)).astype(np.float32))
    logits += b_dec[None, :]
    # stable log_softmax on host
    mx = logits.max(axis=1, keepdims=True)
    lse = mx + np.log(np.exp(logits - mx).sum(axis=1, keepdims=True))
    out = logits - lse
    return out.astype(np.float32)


if __name__ == "__main__":
    # smoke test against reference
    sys.path.insert(0, os.path.dirname(os.path.abspath(__file__)))
    import reference
    t0 = time.time()
    inputs = {k: np.asarray(v) for k, v in reference.setup_inputs().items()}
    exp = np.asarray(reference.reference(**inputs))
    t1 = time.time()
    print(f"reference: {t1-t0:.1f}s")
    act = kernel(**inputs)
    t2 = time.time()
    print(f"kernel: {t2-t1:.1f}s")
    err = np.abs(act - exp)
    rel = err.max() / np.abs(exp).max()
    l2 = np.linalg.norm(act - exp) / np.linalg.norm(exp)
    print(f"max abs err {err.max():.3e}  rel(max) {rel:.3e}  rel L2 {l2:.3e}")


# revision 14
# speedup vs baseline: 1.0102x; 1.0102x over previous
"""KnowledgeRNN Trainium2 kernel: 8-core SPMD.

Device (Bass/Tile, 8 NeuronCores), bf16 matmuls (fp32 PSUM accumulate):
  - Phase A: batched input projections  XP = X @ [Wq1_x | W_ih_x^T]
    (output-dim sharded 8 ways, 768 cols/core), fp32 output.
  - Phase B: decoder  logits = F @ W_dec^T  (vocab sharded 8 ways,
    4000 cols/core), bf16 logits output.
  X (the shared activation matrix) is loaded into SBUF once per core and
  kept resident; W streams through double-buffered tiles; matmuls use the
  full 1024-wide bf16 moving operand (2 PSUM banks per tile).
Host: embedding gather, bias adds, the 2048-step sequential scan glue
(state-dependent matvecs), final log_softmax normalization.
"""
import os
import sys
import time

sys.path.insert(0, '/opt/trn_rl_repo')
sys.path.insert(0, '/opt/trn_rl_repo/concourse')
os.environ.setdefault("MYCRO_LOCAL_CACHE", "1")

import numpy as np
import ml_dtypes

import concourse.bass as bass
import concourse.mybir as mybir
from concourse import bacc, tile, bass_utils
from concourse.tile import add_dep_helper

N_CORES = 8
NTOK, STATE, EMB = 32000, 1024, 1024
QUERY, VALUE, NKB = 256, 512, 10000
SEQ = 2048
QIN = STATE + EMB
DEC_IN = STATE + EMB + VALUE

F32 = mybir.dt.float32
BF16 = mybir.dt.bfloat16
NP_BF16 = ml_dtypes.bfloat16

# enable NTFF tracing for local perf runs (test.py sets this)
_TRACE = os.environ.get("KRNN_TRACE", "") == "1"


def _build_mm_kernel(K, S, N, out_dt=BF16, NW=512):
    """OUT[S,N] = XT^T @ W  (no bias — host adds it).

    Inputs (per core): "xt" [K,S] bf16, "w" [K,N] bf16.
    Output: "out" [S,N] out_dt.
    """
    assert K % 128 == 0 and S % 128 == 0
    KC = K // 128
    ST = S // 128
    nbs = []
    o = 0
    while o < N:
        w = min(NW, N - o)
        nbs.append((o, w))
        o += w
    # Narrowest block first: the first W block's DMA gates the first
    # matmul, so a smaller head block starts the PE earlier. Only safe
    # for deep-K kernels — with small K the narrow head block runs
    # through X faster than the chunk chain can deliver it (measured:
    # K=1024 stalls ~10 µs mid-pipeline; K=2560 is stall-free).
    if K // 128 >= 16:
        nbs.sort(key=lambda t: t[1])

    nc = bacc.Bacc(None, target_bir_lowering=False)
    xt = nc.declare_dram_parameter("xt", [K, S], BF16, isOutput=False)
    wt = nc.declare_dram_parameter("w", [K, N], BF16, isOutput=False)
    out = nc.declare_dram_parameter("out", [S, N], out_dt, isOutput=True)

    xt_v = xt.rearrange("(kb p) s -> p kb s", p=128)
    wt_v = wt.rearrange("(kb p) n -> p kb n", p=128)

    # X resident in SBUF, loaded once as a chained sequence of chunks along
    # S: a small head chunk so the first matmul starts early, then 512-col
    # chunks each DMA-gated on the previous one so they never steal HBM
    # bandwidth from the transfer the PE is actually waiting on.
    if S >= 1024:
        xcols = [256] + [512] * ((S - 512) // 512) + [256]
    else:
        xcols = [S]
    xoffs = [sum(xcols[:i]) for i in range(len(xcols))]

    with tile.TileContext(nc) as tc:
        with (
            tc.tile_pool(name="xres", bufs=1) as xres,
            tc.tile_pool(name="wpool", bufs=2) as wpool,
            tc.tile_pool(name="opool", bufs=3) as opool,
            tc.tile_pool(name="ppool", bufs=3, space="PSUM") as ppool,
        ):
            # Startup bandwidth management: issue W block 0 first, then the
            # small X head chunk; only those two compete for HBM before the
            # first matmul. Later X chunks chain on the previous chunk's
            # DMA; the W block 1 prefetch waits for the mid X chunk.
            wblk0 = wpool.tile([128, KC, NW], BF16, tag="w", name="wblk0")
            nbo0, nbw0 = nbs[0]
            nc.sync.dma_start(out=wblk0[:, :, :nbw0], in_=wt_v[:, :, nbo0:nbo0 + nbw0])

            xch = []
            xdmas = []
            for c, (xo_c, xw_c) in enumerate(zip(xoffs, xcols)):
                xc_t = xres.tile([128, KC, xw_c], BF16, tag=f"x{c}", name=f"x{c}")
                xd = nc.sync.dma_start(
                    out=xc_t[:, :, :], in_=xt_v[:, :, xo_c:xo_c + xw_c]
                )
                if c > 0:
                    add_dep_helper(
                        xd.ins, xdmas[c - 1].ins, sync=True,
                        reason="chain x chunk loads",
                    )
                xch.append(xc_t)
                xdmas.append(xd)

            def x_slice(st):
                pos = st * 128
                for c in range(len(xcols)):
                    if pos < xoffs[c] + xcols[c]:
                        return xch[c], pos - xoffs[c]
                raise AssertionError

            w1_gate = xdmas[min(2, len(xdmas) - 1)]
            for nbi, (nbo, nbw) in enumerate(nbs):
                if nbi == 0:
                    wblk = wblk0
                else:
                    wblk = wpool.tile([128, KC, NW], BF16, tag="w", name=f"wblk{nbi}")
                    wd = nc.sync.dma_start(
                        out=wblk[:, :, :nbw], in_=wt_v[:, :, nbo:nbo + nbw]
                    )
                    if nbi == 1:
                        add_dep_helper(
                            wd.ins, w1_gate.ins, sync=True,
                            reason="hold W prefetch off the startup stream",
                        )
                for st in range(ST):
                    xc, xo = x_slice(st)
                    ps = ppool.tile([128, NW], F32, tag="ps")
                    for kb in range(KC):
                        nc.tensor.matmul(
                            ps[:, :nbw], xc[:, kb, xo:xo + 128], wblk[:, kb, :nbw],
                            start=(kb == 0), stop=(kb == KC - 1),
                        )
                    ot = opool.tile([128, NW], out_dt, tag="o")
                    nc.vector.tensor_copy(out=ot[:, :nbw], in_=ps[:, :nbw])
                    nc.sync.dma_start(
                        out=out[st * 128:(st + 1) * 128, nbo:nbo + nbw],
                        in_=ot[:, :nbw],
                    )
    nc.compile()
    return nc


_KERNEL_CACHE = {}
LAST_EXEC_NS = 0


def _run_mm(key, K, S, N, xts, ws, out_dt=BF16, NW=512):
    global LAST_EXEC_NS
    if key not in _KERNEL_CACHE:
        _KERNEL_CACHE[key] = _build_mm_kernel(K, S, N, out_dt, NW)
    nc = _KERNEL_CACHE[key]
    in_maps = [
        {"xt": np.ascontiguousarray(xts[c]),
         "w": np.ascontiguousarray(ws[c])}
        for c in range(N_CORES)
    ]
    kwargs = {}
    if _TRACE:
        kwargs["trace"] = True
    res = bass_utils.run_bass_kernel_spmd(
        nc, in_maps, core_ids=list(range(N_CORES)), **kwargs
    )
    if res.exec_time_ns:
        LAST_EXEC_NS += res.exec_time_ns
    return res


def kernel(input_ids, enc_W, Wq1, bq1, Wq2, bq2, kb_keys, kb_vals,
           W_ih, b_ih, W_hh, b_hh, W_dec, b_dec):
    input_ids = np.asarray(input_ids)
    enc_W = np.asarray(enc_W, np.float32)
    Wq1 = np.asarray(Wq1, np.float32)
    bq1 = np.asarray(bq1, np.float32)
    Wq2 = np.asarray(Wq2, np.float32)
    bq2 = np.asarray(bq2, np.float32)
    kb_keys = np.asarray(kb_keys, np.float32)
    kb_vals = np.asarray(kb_vals, np.float32)
    W_ih = np.asarray(W_ih, np.float32)
    b_ih = np.asarray(b_ih, np.float32)
    W_hh = np.asarray(W_hh, np.float32)
    b_hh = np.asarray(b_hh, np.float32)
    W_dec = np.asarray(W_dec, np.float32)
    b_dec = np.asarray(b_dec, np.float32)

    # ---- embedding gather (host glue) ----
    emb = enc_W[input_ids]                      # [S, EMB] fp32
    X_T16 = np.ascontiguousarray(emb.T.astype(NP_BF16))   # [EMB, S] bf16

    # ---- Phase A on device: XP = X @ [Wq1_x | W_ih_x^T]; bias added on host
    Wq1_x = Wq1[STATE:, :]                      # [1024, 2048]
    W_ih_xT = np.ascontiguousarray(W_ih[:, :EMB].T)   # [1024, 4096]
    PROJ = np.concatenate([Wq1_x, W_ih_xT], axis=1).astype(NP_BF16)  # [1024, 6144]
    BIAS = np.concatenate([bq1, b_ih + b_hh]).astype(np.float32)     # [6144]
    NSH = 6144 // N_CORES                              # 768
    ws = [PROJ[:, c * NSH:(c + 1) * NSH] for c in range(N_CORES)]
    xts = [X_T16] * N_CORES
    resA = _run_mm("A", EMB, SEQ, NSH, xts, ws, out_dt=F32)
    XP = np.concatenate([resA.results[c]["out"] for c in range(N_CORES)], axis=1)
    XP += BIAS[None, :]
    xq_pre = XP[:, :2048]                        # [S, 2048]  (= x@Wq1_x + bq1)
    xg_pre = XP[:, 2048:]                        # [S, 4096]  (= x@W_ih_x^T + b_ih + b_hh)

    # ---- host sequential scan (glue around device-precomputed projections) ----
    Wq1_h = np.ascontiguousarray(Wq1[:STATE, :])       # [1024, 2048]
    HXW = np.concatenate([Wq1_h, W_hh.T], axis=1)      # [1024, 2048+4096]
    HXW = np.ascontiguousarray(HXW)
    W_ihvT = np.ascontiguousarray(W_ih[:, EMB:].T)     # [512, 4096]
    kb_keys_c = np.ascontiguousarray(kb_keys)
    kb_vals_c = np.ascontiguousarray(kb_vals)
    Wq2_c = np.ascontiguousarray(Wq2)

    hx = np.zeros(STATE, np.float32)
    cx = np.zeros(STATE, np.float32)
    lstm_states = np.empty((SEQ, STATE), np.float32)
    kb_out = np.empty((SEQ, VALUE), np.float32)
    _t0 = time.time()
    for t in range(SEQ):
        if t % 512 == 0:
            print(f"[kernel] scan step {t} ({time.time()-_t0:.1f}s)", flush=True)
        lstm_states[t] = hx
        hp = hx @ HXW                                  # [6144]
        qh = np.tanh(hp[:2048] + xq_pre[t])
        q = qh @ Wq2_c + bq2                           # [256]
        sc = kb_keys_c @ q                             # [NKB]
        sc -= sc.max()
        u = np.exp(sc)
        attn = u / u.sum()
        val = attn @ kb_vals_c                         # [512]
        kb_out[t] = val
        gates = xg_pre[t] + val @ W_ihvT + hp[2048:]   # [4096]
        i_g = gates[:1024]
        f_g = gates[1024:2048]
        g_g = gates[2048:3072]
        o_g = gates[3072:]
        sig_i = 1.0 / (1.0 + np.exp(-i_g))
        sig_f = 1.0 / (1.0 + np.exp(-f_g))
        sig_o = 1.0 / (1.0 + np.exp(-o_g))
        cx = sig_f * cx + sig_i * np.tanh(g_g)
        hx = sig_o * np.tanh(cx)

    # ---- Phase B on device: decoder (host adds bias + log_softmax) ----
    F = np.concatenate([emb, kb_out, lstm_states], axis=1)   # [S, 2560]
    F_T16 = np.ascontiguousarray(F.T.astype(NP_BF16))        # [2560, S] bf16
    VSH = NTOK // N_CORES                                    # 4000
    wdt16 = np.ascontiguousarray(W_dec.T.astype(NP_BF16))    # [2560, 32000]
    ws_b = [np.ascontiguousarray(wdt16[:, c * VSH:(c + 1) * VSH]) for c in range(N_CORES)]
    xts_b = [F_T16] * N_CORES
    resB = _run_mm("B", DEC_IN, SEQ, VSH, xts_b, ws_b, out_dt=BF16)

    logits = np.concatenate(
        [resB.results[c]["out"].astype(np.float32) for c in range(N_CORES)], axis=1
    )
    if os.environ.get("KRNN_DUMP_CACHE"):
        np.savez("/tmp/krnn_f_cache.npz",
                 F_T16=F_T16.view(np.uint16), wdt16=wdt16.view(np.uint16),
                 logits=(F @ W_dec.T.astype(np.float32)).astype(np.float32))
    logits += b_dec[None, :]
    # stable log_softmax on host
    mx = logits.max(axis=1, keepdims=True)
    lse = mx + np.log(np.exp(logits - mx).sum(axis=1, keepdims=True))
    out = logits - lse
    return out.astype(np.float32)


if __name__ == "__main__":
    # smoke test against reference
    sys.path.insert(0, os.path.dirname(os.path.abspath(__file__)))
    import reference
    t0 = time.time()
    inputs = {k: np.asarray(v) for k, v in reference.setup_inputs().items()}
    exp = np.asarray(reference.reference(**inputs))
    t1 = time.time()
    print(f"reference: {t1-t0:.1f}s")
    act = kernel(**inputs)
    t2 = time.time()
    print(f"kernel: {t2-t1:.1f}s")
    err = np.abs(act - exp)
    rel = err.max() / np.abs(exp).max()
    l2 = np.linalg.norm(act - exp) / np.linalg.norm(exp)
    print(f"max abs err {err.max():.3e}  rel(max) {rel:.3e}  rel L2 {l2:.3e}")


# revision 16
# speedup vs baseline: 1.0202x; 1.0099x over previous
"""KnowledgeRNN Trainium2 kernel: 8-core SPMD.

Device (Bass/Tile, 8 NeuronCores), bf16 matmuls (fp32 PSUM accumulate):
  - Phase A: batched input projections  XP = X @ [Wq1_x | W_ih_x^T]
    (output-dim sharded 8 ways, 768 cols/core), fp32 output.
  - Phase B: decoder  logits = F @ W_dec^T  (vocab sharded 8 ways,
    4000 cols/core), bf16 logits output.
  X (the shared activation matrix) is loaded into SBUF once per core and
  kept resident; W streams through double-buffered tiles; matmuls use the
  full 1024-wide bf16 moving operand (2 PSUM banks per tile).
Host: embedding gather, bias adds, the 2048-step sequential scan glue
(state-dependent matvecs), final log_softmax normalization.
"""
import os
import sys
import time

sys.path.insert(0, '/opt/trn_rl_repo')
sys.path.insert(0, '/opt/trn_rl_repo/concourse')
os.environ.setdefault("MYCRO_LOCAL_CACHE", "1")

import numpy as np
import ml_dtypes

import concourse.bass as bass
import concourse.mybir as mybir
from concourse import bacc, tile, bass_utils
from concourse.tile import add_dep_helper

N_CORES = 8
NTOK, STATE, EMB = 32000, 1024, 1024
QUERY, VALUE, NKB = 256, 512, 10000
SEQ = 2048
QIN = STATE + EMB
DEC_IN = STATE + EMB + VALUE

F32 = mybir.dt.float32
BF16 = mybir.dt.bfloat16
NP_BF16 = ml_dtypes.bfloat16

# enable NTFF tracing for local perf runs (test.py sets this)
_TRACE = os.environ.get("KRNN_TRACE", "") == "1"


def _build_mm_kernel(K, S, N, out_dt=BF16, NW=512):
    """OUT[S,N] = XT^T @ W  (no bias — host adds it).

    Inputs (per core): "xt" [K,S] bf16, "w" [K,N] bf16.
    Output: "out" [S,N] out_dt.
    """
    assert K % 128 == 0 and S % 128 == 0
    KC = K // 128
    ST = S // 128
    nbs = []
    o = 0
    while o < N:
        w = min(NW, N - o)
        nbs.append((o, w))
        o += w
    # Narrowest block first: the first W block's DMA gates the first
    # matmul, so a smaller head block starts the PE earlier. Only safe
    # for deep-K kernels — with small K the narrow head block runs
    # through X faster than the chunk chain can deliver it (measured:
    # K=1024 stalls ~10 µs mid-pipeline; K=2560 is stall-free).
    if K // 128 >= 16:
        nbs.sort(key=lambda t: t[1])

    nc = bacc.Bacc(None, target_bir_lowering=False)
    xt = nc.declare_dram_parameter("xt", [K, S], BF16, isOutput=False)
    wt = nc.declare_dram_parameter("w", [K, N], BF16, isOutput=False)
    out = nc.declare_dram_parameter("out", [S, N], out_dt, isOutput=True)

    xt_v = xt.rearrange("(kb p) s -> p kb s", p=128)
    wt_v = wt.rearrange("(kb p) n -> p kb n", p=128)

    # X resident in SBUF, loaded once as a chained sequence of chunks along
    # S: a small head chunk so the first matmul starts early, then 512-col
    # chunks each DMA-gated on the previous one so they never steal HBM
    # bandwidth from the transfer the PE is actually waiting on.
    if S >= 1024:
        xcols = [256] + [512] * ((S - 512) // 512) + [256]
    else:
        xcols = [S]
    xoffs = [sum(xcols[:i]) for i in range(len(xcols))]

    with tile.TileContext(nc) as tc:
        with (
            tc.tile_pool(name="xres", bufs=1) as xres,
            tc.tile_pool(name="wpool", bufs=2) as wpool,
            tc.tile_pool(name="opool", bufs=3) as opool,
            tc.tile_pool(name="ppool", bufs=3, space="PSUM") as ppool,
        ):
            # Startup bandwidth management: issue W block 0 first, then the
            # small X head chunk; only those two compete for HBM before the
            # first matmul. Later X chunks chain on the previous chunk's
            # DMA; the W block 1 prefetch waits for the mid X chunk.
            wblk0 = wpool.tile([128, KC, NW], BF16, tag="w", name="wblk0")
            nbo0, nbw0 = nbs[0]
            nc.sync.dma_start(out=wblk0[:, :, :nbw0], in_=wt_v[:, :, nbo0:nbo0 + nbw0])

            xch = []
            xdmas = []
            for c, (xo_c, xw_c) in enumerate(zip(xoffs, xcols)):
                xc_t = xres.tile([128, KC, xw_c], BF16, tag=f"x{c}", name=f"x{c}")
                xd = nc.sync.dma_start(
                    out=xc_t[:, :, :], in_=xt_v[:, :, xo_c:xo_c + xw_c]
                )
                if c > 0:
                    add_dep_helper(
                        xd.ins, xdmas[c - 1].ins, sync=True,
                        reason="chain x chunk loads",
                    )
                xch.append(xc_t)
                xdmas.append(xd)

            def x_slice(st):
                pos = st * 128
                for c in range(len(xcols)):
                    if pos < xoffs[c] + xcols[c]:
                        return xch[c], pos - xoffs[c]
                raise AssertionError

            w1_gate = xdmas[min(2, len(xdmas) - 1)]
            for nbi, (nbo, nbw) in enumerate(nbs):
                if nbi == 0:
                    wblk = wblk0
                else:
                    wblk = wpool.tile([128, KC, NW], BF16, tag="w", name=f"wblk{nbi}")
                    wd = nc.sync.dma_start(
                        out=wblk[:, :, :nbw], in_=wt_v[:, :, nbo:nbo + nbw]
                    )
                    if nbi == 1:
                        add_dep_helper(
                            wd.ins, w1_gate.ins, sync=True,
                            reason="hold W prefetch off the startup stream",
                        )
                for st in range(ST):
                    xc, xo = x_slice(st)
                    ps = ppool.tile([128, NW], F32, tag="ps")
                    for kb in range(KC):
                        nc.tensor.matmul(
                            ps[:, :nbw], xc[:, kb, xo:xo + 128], wblk[:, kb, :nbw],
                            start=(kb == 0), stop=(kb == KC - 1),
                        )
                    ot = opool.tile([128, NW], out_dt, tag="o")
                    nc.vector.tensor_copy(out=ot[:, :nbw], in_=ps[:, :nbw])
                    nc.sync.dma_start(
                        out=out[st * 128:(st + 1) * 128, nbo:nbo + nbw],
                        in_=ot[:, :nbw],
                    )
    nc.compile()
    return nc


_KERNEL_CACHE = {}
LAST_EXEC_NS = 0


def _run_mm(key, K, S, N, xts, ws, out_dt=BF16, NW=512):
    global LAST_EXEC_NS
    if key not in _KERNEL_CACHE:
        _KERNEL_CACHE[key] = _build_mm_kernel(K, S, N, out_dt, NW)
    nc = _KERNEL_CACHE[key]
    in_maps = [
        {"xt": np.ascontiguousarray(xts[c]),
         "w": np.ascontiguousarray(ws[c])}
        for c in range(N_CORES)
    ]
    kwargs = {}
    if _TRACE:
        kwargs["trace"] = True
    res = bass_utils.run_bass_kernel_spmd(
        nc, in_maps, core_ids=list(range(N_CORES)), **kwargs
    )
    if res.exec_time_ns:
        LAST_EXEC_NS += res.exec_time_ns
    return res


def kernel(input_ids, enc_W, Wq1, bq1, Wq2, bq2, kb_keys, kb_vals,
           W_ih, b_ih, W_hh, b_hh, W_dec, b_dec):
    input_ids = np.asarray(input_ids)
    enc_W = np.asarray(enc_W, np.float32)
    Wq1 = np.asarray(Wq1, np.float32)
    bq1 = np.asarray(bq1, np.float32)
    Wq2 = np.asarray(Wq2, np.float32)
    bq2 = np.asarray(bq2, np.float32)
    kb_keys = np.asarray(kb_keys, np.float32)
    kb_vals = np.asarray(kb_vals, np.float32)
    W_ih = np.asarray(W_ih, np.float32)
    b_ih = np.asarray(b_ih, np.float32)
    W_hh = np.asarray(W_hh, np.float32)
    b_hh = np.asarray(b_hh, np.float32)
    W_dec = np.asarray(W_dec, np.float32)
    b_dec = np.asarray(b_dec, np.float32)

    # ---- embedding gather (host glue) ----
    emb = enc_W[input_ids]                      # [S, EMB] fp32
    X_T16 = np.ascontiguousarray(emb.T.astype(NP_BF16))   # [EMB, S] bf16

    # ---- Phase A on device: XP = X @ [Wq1_x | W_ih_x^T]; bias added on host
    Wq1_x = Wq1[STATE:, :]                      # [1024, 2048]
    W_ih_xT = np.ascontiguousarray(W_ih[:, :EMB].T)   # [1024, 4096]
    PROJ = np.concatenate([Wq1_x, W_ih_xT], axis=1).astype(NP_BF16)  # [1024, 6144]
    BIAS = np.concatenate([bq1, b_ih + b_hh]).astype(np.float32)     # [6144]
    NSH = 6144 // N_CORES                              # 768
    ws = [PROJ[:, c * NSH:(c + 1) * NSH] for c in range(N_CORES)]
    xts = [X_T16] * N_CORES
    resA = _run_mm("A", EMB, SEQ, NSH, xts, ws, out_dt=F32)
    XP = np.concatenate([resA.results[c]["out"] for c in range(N_CORES)], axis=1)
    XP += BIAS[None, :]
    xq_pre = XP[:, :2048]                        # [S, 2048]  (= x@Wq1_x + bq1)
    xg_pre = XP[:, 2048:]                        # [S, 4096]  (= x@W_ih_x^T + b_ih + b_hh)

    # ---- host sequential scan (glue around device-precomputed projections) ----
    Wq1_h = np.ascontiguousarray(Wq1[:STATE, :])       # [1024, 2048]
    HXW = np.concatenate([Wq1_h, W_hh.T], axis=1)      # [1024, 2048+4096]
    HXW = np.ascontiguousarray(HXW)
    W_ihvT = np.ascontiguousarray(W_ih[:, EMB:].T)     # [512, 4096]
    kb_keys_c = np.ascontiguousarray(kb_keys)
    kb_vals_c = np.ascontiguousarray(kb_vals)
    Wq2_c = np.ascontiguousarray(Wq2)

    hx = np.zeros(STATE, np.float32)
    cx = np.zeros(STATE, np.float32)
    lstm_states = np.empty((SEQ, STATE), np.float32)
    kb_out = np.empty((SEQ, VALUE), np.float32)
    _t0 = time.time()
    for t in range(SEQ):
        if t % 512 == 0:
            print(f"[kernel] scan step {t} ({time.time()-_t0:.1f}s)", flush=True)
        lstm_states[t] = hx
        hp = hx @ HXW                                  # [6144]
        qh = np.tanh(hp[:2048] + xq_pre[t])
        q = qh @ Wq2_c + bq2                           # [256]
        sc = kb_keys_c @ q                             # [NKB]
        sc -= sc.max()
        u = np.exp(sc)
        attn = u / u.sum()
        val = attn @ kb_vals_c                         # [512]
        kb_out[t] = val
        gates = xg_pre[t] + val @ W_ihvT + hp[2048:]   # [4096]
        i_g = gates[:1024]
        f_g = gates[1024:2048]
        g_g = gates[2048:3072]
        o_g = gates[3072:]
        sig_i = 1.0 / (1.0 + np.exp(-i_g))
        sig_f = 1.0 / (1.0 + np.exp(-f_g))
        sig_o = 1.0 / (1.0 + np.exp(-o_g))
        cx = sig_f * cx + sig_i * np.tanh(g_g)
        hx = sig_o * np.tanh(cx)

    # ---- Phase B on device: decoder (host adds bias + log_softmax) ----
    F = np.concatenate([emb, kb_out, lstm_states], axis=1)   # [S, 2560]
    F_T16 = np.ascontiguousarray(F.T.astype(NP_BF16))        # [2560, S] bf16
    VSH = NTOK // N_CORES                                    # 4000
    wdt16 = np.ascontiguousarray(W_dec.T.astype(NP_BF16))    # [2560, 32000]
    ws_b = [np.ascontiguousarray(wdt16[:, c * VSH:(c + 1) * VSH]) for c in range(N_CORES)]
    xts_b = [F_T16] * N_CORES
    resB = _run_mm("B", DEC_IN, SEQ, VSH, xts_b, ws_b, out_dt=BF16)

    logits = np.concatenate(
        [resB.results[c]["out"].astype(np.float32) for c in range(N_CORES)], axis=1
    )
    if os.environ.get("KRNN_DUMP_CACHE"):
        np.savez("/tmp/krnn_f_cache.npz",
                 F_T16=F_T16.view(np.uint16), wdt16=wdt16.view(np.uint16),
                 logits=(F @ W_dec.T.astype(np.float32)).astype(np.float32))
    logits += b_dec[None, :]
    # stable log_softmax on host
    mx = logits.max(axis=1, keepdims=True)
    lse = mx + np.log(np.exp(logits - mx).sum(axis=1, keepdims=True))
    out = logits - lse
    return out.astype(np.float32)


if __name__ == "__main__":
    # smoke test against reference
    sys.path.insert(0, os.path.dirname(os.path.abspath(__file__)))
    import reference
    t0 = time.time()
    inputs = {k: np.asarray(v) for k, v in reference.setup_inputs().items()}
    exp = np.asarray(reference.reference(**inputs))
    t1 = time.time()
    print(f"reference: {t1-t0:.1f}s")
    act = kernel(**inputs)
    t2 = time.time()
    print(f"kernel: {t2-t1:.1f}s")
    err = np.abs(act - exp)
    rel = err.max() / np.abs(exp).max()
    l2 = np.linalg.norm(act - exp) / np.linalg.norm(exp)
    print(f"max abs err {err.max():.3e}  rel(max) {rel:.3e}  rel L2 {l2:.3e}")


# revision 17
# speedup vs baseline: 1.0237x; 1.0033x over previous
"""KnowledgeRNN Trainium2 kernel: 8-core SPMD.

Device (Bass/Tile, 8 NeuronCores), bf16 matmuls (fp32 PSUM accumulate):
  - Phase A: batched input projections  XP = X @ [Wq1_x | W_ih_x^T]
    (output-dim sharded 8 ways, 768 cols/core), fp32 output.
  - Phase B: decoder  logits = F @ W_dec^T  (vocab sharded 8 ways,
    4000 cols/core), bf16 logits output.
  X (the shared activation matrix) is loaded into SBUF once per core and
  kept resident; W streams through double-buffered tiles; matmuls use the
  full 1024-wide bf16 moving operand (2 PSUM banks per tile).
Host: embedding gather, bias adds, the 2048-step sequential scan glue
(state-dependent matvecs), final log_softmax normalization.
"""
import os
import sys
import time

sys.path.insert(0, '/opt/trn_rl_repo')
sys.path.insert(0, '/opt/trn_rl_repo/concourse')
os.environ.setdefault("MYCRO_LOCAL_CACHE", "1")

import numpy as np
import ml_dtypes

import concourse.bass as bass
import concourse.mybir as mybir
from concourse import bacc, tile, bass_utils
from concourse.tile import add_dep_helper

N_CORES = 8
NTOK, STATE, EMB = 32000, 1024, 1024
QUERY, VALUE, NKB = 256, 512, 10000
SEQ = 2048
QIN = STATE + EMB
DEC_IN = STATE + EMB + VALUE

F32 = mybir.dt.float32
BF16 = mybir.dt.bfloat16
NP_BF16 = ml_dtypes.bfloat16

# enable NTFF tracing for local perf runs (test.py sets this)
_TRACE = os.environ.get("KRNN_TRACE", "") == "1"


def _build_mm_kernel(K, S, N, out_dt=BF16, NW=512):
    """OUT[S,N] = XT^T @ W  (no bias — host adds it).

    Inputs (per core): "xt" [K,S] bf16, "w" [K,N] bf16.
    Output: "out" [S,N] out_dt.
    """
    assert K % 128 == 0 and S % 128 == 0
    KC = K // 128
    ST = S // 128
    nbs = []
    o = 0
    while o < N:
        w = min(NW, N - o)
        nbs.append((o, w))
        o += w
    # Narrowest block first: the first W block's DMA gates the first
    # matmul, so a smaller head block starts the PE earlier. Only safe
    # for deep-K kernels — with small K the narrow head block runs
    # through X faster than the chunk chain can deliver it (measured:
    # K=1024 stalls ~10 µs mid-pipeline; K=2560 is stall-free).
    if K // 128 >= 16:
        nbs.sort(key=lambda t: t[1])

    nc = bacc.Bacc(None, target_bir_lowering=False)
    xt = nc.declare_dram_parameter("xt", [K, S], BF16, isOutput=False)
    wt = nc.declare_dram_parameter("w", [K, N], BF16, isOutput=False)
    out = nc.declare_dram_parameter("out", [S, N], out_dt, isOutput=True)

    xt_v = xt.rearrange("(kb p) s -> p kb s", p=128)
    wt_v = wt.rearrange("(kb p) n -> p kb n", p=128)

    # X resident in SBUF, loaded once as a chained sequence of chunks along
    # S: a small head chunk so the first matmul starts early, then 512-col
    # chunks each DMA-gated on the previous one so they never steal HBM
    # bandwidth from the transfer the PE is actually waiting on.
    if S >= 1024:
        xcols = [256] + [512] * ((S - 512) // 512) + [256]
    else:
        xcols = [S]
    xoffs = [sum(xcols[:i]) for i in range(len(xcols))]

    # Shallow-K (Phase A) kernels write fp32 output at a high average rate;
    # deeper out/PSUM buffering rides out transient HBM write backpressure
    # that otherwise backs up through DVE into the PE (measured ~2 µs of
    # mid-pipeline PE gaps). Deep-K (Phase B) is gap-free at 3/3.
    deep = K // 128 >= 16
    with tile.TileContext(nc) as tc:
        with (
            tc.tile_pool(name="xres", bufs=1) as xres,
            tc.tile_pool(name="wpool", bufs=2) as wpool,
            tc.tile_pool(name="opool", bufs=3 if deep else 6) as opool,
            tc.tile_pool(name="ppool", bufs=3 if deep else 4, space="PSUM") as ppool,
        ):
            # Startup bandwidth management: issue W block 0 first, then the
            # small X head chunk; only those two compete for HBM before the
            # first matmul. Later X chunks chain on the previous chunk's
            # DMA; the W block 1 prefetch waits for the mid X chunk.
            wblk0 = wpool.tile([128, KC, NW], BF16, tag="w", name="wblk0")
            nbo0, nbw0 = nbs[0]
            nc.sync.dma_start(out=wblk0[:, :, :nbw0], in_=wt_v[:, :, nbo0:nbo0 + nbw0])

            xch = []
            xdmas = []
            for c, (xo_c, xw_c) in enumerate(zip(xoffs, xcols)):
                xc_t = xres.tile([128, KC, xw_c], BF16, tag=f"x{c}", name=f"x{c}")
                xd = nc.sync.dma_start(
                    out=xc_t[:, :, :], in_=xt_v[:, :, xo_c:xo_c + xw_c]
                )
                if c > 0:
                    add_dep_helper(
                        xd.ins, xdmas[c - 1].ins, sync=True,
                        reason="chain x chunk loads",
                    )
                xch.append(xc_t)
                xdmas.append(xd)

            def x_slice(st):
                pos = st * 128
                for c in range(len(xcols)):
                    if pos < xoffs[c] + xcols[c]:
                        return xch[c], pos - xoffs[c]
                raise AssertionError

            w1_gate = xdmas[min(2, len(xdmas) - 1)]
            for nbi, (nbo, nbw) in enumerate(nbs):
                if nbi == 0:
                    wblk = wblk0
                else:
                    wblk = wpool.tile([128, KC, NW], BF16, tag="w", name=f"wblk{nbi}")
                    wd = nc.sync.dma_start(
                        out=wblk[:, :, :nbw], in_=wt_v[:, :, nbo:nbo + nbw]
                    )
                    if nbi == 1:
                        add_dep_helper(
                            wd.ins, w1_gate.ins, sync=True,
                            reason="hold W prefetch off the startup stream",
                        )
                for st in range(ST):
                    xc, xo = x_slice(st)
                    ps = ppool.tile([128, NW], F32, tag="ps")
                    for kb in range(KC):
                        nc.tensor.matmul(
                            ps[:, :nbw], xc[:, kb, xo:xo + 128], wblk[:, kb, :nbw],
                            start=(kb == 0), stop=(kb == KC - 1),
                        )
                    ot = opool.tile([128, NW], out_dt, tag="o")
                    nc.vector.tensor_copy(out=ot[:, :nbw], in_=ps[:, :nbw])
                    nc.sync.dma_start(
                        out=out[st * 128:(st + 1) * 128, nbo:nbo + nbw],
                        in_=ot[:, :nbw],
                    )
    nc.compile()
    return nc


_KERNEL_CACHE = {}
LAST_EXEC_NS = 0


def _run_mm(key, K, S, N, xts, ws, out_dt=BF16, NW=512):
    global LAST_EXEC_NS
    if key not in _KERNEL_CACHE:
        _KERNEL_CACHE[key] = _build_mm_kernel(K, S, N, out_dt, NW)
    nc = _KERNEL_CACHE[key]
    in_maps = [
        {"xt": np.ascontiguousarray(xts[c]),
         "w": np.ascontiguousarray(ws[c])}
        for c in range(N_CORES)
    ]
    kwargs = {}
    if _TRACE:
        kwargs["trace"] = True
    res = bass_utils.run_bass_kernel_spmd(
        nc, in_maps, core_ids=list(range(N_CORES)), **kwargs
    )
    if res.exec_time_ns:
        LAST_EXEC_NS += res.exec_time_ns
    return res


def kernel(input_ids, enc_W, Wq1, bq1, Wq2, bq2, kb_keys, kb_vals,
           W_ih, b_ih, W_hh, b_hh, W_dec, b_dec):
    input_ids = np.asarray(input_ids)
    enc_W = np.asarray(enc_W, np.float32)
    Wq1 = np.asarray(Wq1, np.float32)
    bq1 = np.asarray(bq1, np.float32)
    Wq2 = np.asarray(Wq2, np.float32)
    bq2 = np.asarray(bq2, np.float32)
    kb_keys = np.asarray(kb_keys, np.float32)
    kb_vals = np.asarray(kb_vals, np.float32)
    W_ih = np.asarray(W_ih, np.float32)
    b_ih = np.asarray(b_ih, np.float32)
    W_hh = np.asarray(W_hh, np.float32)
    b_hh = np.asarray(b_hh, np.float32)
    W_dec = np.asarray(W_dec, np.float32)
    b_dec = np.asarray(b_dec, np.float32)

    # ---- embedding gather (host glue) ----
    emb = enc_W[input_ids]                      # [S, EMB] fp32
    X_T16 = np.ascontiguousarray(emb.T.astype(NP_BF16))   # [EMB, S] bf16

    # ---- Phase A on device: XP = X @ [Wq1_x | W_ih_x^T]; bias added on host
    Wq1_x = Wq1[STATE:, :]                      # [1024, 2048]
    W_ih_xT = np.ascontiguousarray(W_ih[:, :EMB].T)   # [1024, 4096]
    PROJ = np.concatenate([Wq1_x, W_ih_xT], axis=1).astype(NP_BF16)  # [1024, 6144]
    BIAS = np.concatenate([bq1, b_ih + b_hh]).astype(np.float32)     # [6144]
    NSH = 6144 // N_CORES                              # 768
    ws = [PROJ[:, c * NSH:(c + 1) * NSH] for c in range(N_CORES)]
    xts = [X_T16] * N_CORES
    resA = _run_mm("A", EMB, SEQ, NSH, xts, ws, out_dt=F32)
    XP = np.concatenate([resA.results[c]["out"] for c in range(N_CORES)], axis=1)
    XP += BIAS[None, :]
    xq_pre = XP[:, :2048]                        # [S, 2048]  (= x@Wq1_x + bq1)
    xg_pre = XP[:, 2048:]                        # [S, 4096]  (= x@W_ih_x^T + b_ih + b_hh)

    # ---- host sequential scan (glue around device-precomputed projections) ----
    Wq1_h = np.ascontiguousarray(Wq1[:STATE, :])       # [1024, 2048]
    HXW = np.concatenate([Wq1_h, W_hh.T], axis=1)      # [1024, 2048+4096]
    HXW = np.ascontiguousarray(HXW)
    W_ihvT = np.ascontiguousarray(W_ih[:, EMB:].T)     # [512, 4096]
    kb_keys_c = np.ascontiguousarray(kb_keys)
    kb_vals_c = np.ascontiguousarray(kb_vals)
    Wq2_c = np.ascontiguousarray(Wq2)

    hx = np.zeros(STATE, np.float32)
    cx = np.zeros(STATE, np.float32)
    lstm_states = np.empty((SEQ, STATE), np.float32)
    kb_out = np.empty((SEQ, VALUE), np.float32)
    _t0 = time.time()
    for t in range(SEQ):
        if t % 512 == 0:
            print(f"[kernel] scan step {t} ({time.time()-_t0:.1f}s)", flush=True)
        lstm_states[t] = hx
        hp = hx @ HXW                                  # [6144]
        qh = np.tanh(hp[:2048] + xq_pre[t])
        q = qh @ Wq2_c + bq2                           # [256]
        sc = kb_keys_c @ q                             # [NKB]
        sc -= sc.max()
        u = np.exp(sc)
        attn = u / u.sum()
        val = attn @ kb_vals_c                         # [512]
        kb_out[t] = val
        gates = xg_pre[t] + val @ W_ihvT + hp[2048:]   # [4096]
        i_g = gates[:1024]
        f_g = gates[1024:2048]
        g_g = gates[2048:3072]
        o_g = gates[3072:]
        sig_i = 1.0 / (1.0 + np.exp(-i_g))
        sig_f = 1.0 / (1.0 + np.exp(-f_g))
        sig_o = 1.0 / (1.0 + np.exp(-o_g))
        cx = sig_f * cx + sig_i * np.tanh(g_g)
        hx = sig_o * np.tanh(cx)

    # ---- Phase B on device: decoder (host adds bias + log_softmax) ----
    F = np.concatenate([emb, kb_out, lstm_states], axis=1)   # [S, 2560]
    F_T16 = np.ascontiguousarray(F.T.astype(NP_BF16))        # [2560, S] bf16
    VSH = NTOK // N_CORES                                    # 4000
    wdt16 = np.ascontiguousarray(W_dec.T.astype(NP_BF16))    # [2560, 32000]
    ws_b = [np.ascontiguousarray(wdt16[:, c * VSH:(c + 1) * VSH]) for c in range(N_CORES)]
    xts_b = [F_T16] * N_CORES
    resB = _run_mm("B", DEC_IN, SEQ, VSH, xts_b, ws_b, out_dt=BF16)

    logits = np.concatenate(
        [resB.results[c]["out"].astype(np.float32) for c in range(N_CORES)], axis=1
    )
    if os.environ.get("KRNN_DUMP_CACHE"):
        np.savez("/tmp/krnn_f_cache.npz",
                 F_T16=F_T16.view(np.uint16), wdt16=wdt16.view(np.uint16),
                 logits=(F @ W_dec.T.astype(np.float32)).astype(np.float32))
    logits += b_dec[None, :]
    # stable log_softmax on host
    mx = logits.max(axis=1, keepdims=True)
    lse = mx + np.log(np.exp(logits - mx).sum(axis=1, keepdims=True))
    out = logits - lse
    return out.astype(np.float32)


if __name__ == "__main__":
    # smoke test against reference
    sys.path.insert(0, os.path.dirname(os.path.abspath(__file__)))
    import reference
    t0 = time.time()
    inputs = {k: np.asarray(v) for k, v in reference.setup_inputs().items()}
    exp = np.asarray(reference.reference(**inputs))
    t1 = time.time()
    print(f"reference: {t1-t0:.1f}s")
    act = kernel(**inputs)
    t2 = time.time()
    print(f"kernel: {t2-t1:.1f}s")
    err = np.abs(act - exp)
    rel = err.max() / np.abs(exp).max()
    l2 = np.linalg.norm(act - exp) / np.linalg.norm(exp)
    print(f"max abs err {err.max():.3e}  rel(max) {rel:.3e}  rel L2 {l2:.3e}")


# revision 19
# speedup vs baseline: 1.0248x; 1.0011x over previous
"""KnowledgeRNN Trainium2 kernel: 8-core SPMD.

Device (Bass/Tile, 8 NeuronCores), bf16 matmuls (fp32 PSUM accumulate):
  - Phase A: batched input projections  XP = X @ [Wq1_x | W_ih_x^T]
    (output-dim sharded 8 ways, 768 cols/core), fp32 output.
  - Phase B: decoder  logits = F @ W_dec^T  (vocab sharded 8 ways,
    4000 cols/core), bf16 logits output.
  X (the shared activation matrix) is loaded into SBUF once per core and
  kept resident; W streams through double-buffered tiles; matmuls use the
  full 1024-wide bf16 moving operand (2 PSUM banks per tile).
Host: embedding gather, bias adds, the 2048-step sequential scan glue
(state-dependent matvecs), final log_softmax normalization.
"""
import os
import sys
import time

sys.path.insert(0, '/opt/trn_rl_repo')
sys.path.insert(0, '/opt/trn_rl_repo/concourse')
os.environ.setdefault("MYCRO_LOCAL_CACHE", "1")

import numpy as np
import ml_dtypes

import concourse.bass as bass
import concourse.mybir as mybir
from concourse import bacc, tile, bass_utils
from concourse.tile import add_dep_helper

N_CORES = 8
NTOK, STATE, EMB = 32000, 1024, 1024
QUERY, VALUE, NKB = 256, 512, 10000
SEQ = 2048
QIN = STATE + EMB
DEC_IN = STATE + EMB + VALUE

F32 = mybir.dt.float32
BF16 = mybir.dt.bfloat16
NP_BF16 = ml_dtypes.bfloat16

# enable NTFF tracing for local perf runs (test.py sets this)
_TRACE = os.environ.get("KRNN_TRACE", "") == "1"


def _build_mm_kernel(K, S, N, out_dt=BF16, NW=512):
    """OUT[S,N] = XT^T @ W  (no bias — host adds it).

    Inputs (per core): "xt" [K,S] bf16, "w" [K,N] bf16.
    Output: "out" [S,N] out_dt.
    """
    assert K % 128 == 0 and S % 128 == 0
    KC = K // 128
    ST = S // 128
    nbs = []
    o = 0
    while o < N:
        w = min(NW, N - o)
        nbs.append((o, w))
        o += w
    # Narrowest block first: the first W block's DMA gates the first
    # matmul, so a smaller head block starts the PE earlier. Only safe
    # for deep-K kernels — with small K the narrow head block runs
    # through X faster than the chunk chain can deliver it (measured:
    # K=1024 stalls ~10 µs mid-pipeline; K=2560 is stall-free).
    if K // 128 >= 16:
        nbs.sort(key=lambda t: t[1])

    nc = bacc.Bacc(None, target_bir_lowering=False)
    xt = nc.declare_dram_parameter("xt", [K, S], BF16, isOutput=False)
    wt = nc.declare_dram_parameter("w", [K, N], BF16, isOutput=False)
    out = nc.declare_dram_parameter("out", [S, N], out_dt, isOutput=True)

    xt_v = xt.rearrange("(kb p) s -> p kb s", p=128)
    wt_v = wt.rearrange("(kb p) n -> p kb n", p=128)

    # X resident in SBUF, loaded once as a chained sequence of chunks along
    # S: a small head chunk so the first matmul starts early, then 512-col
    # chunks each DMA-gated on the previous one so they never steal HBM
    # bandwidth from the transfer the PE is actually waiting on.
    if S >= 1024:
        xcols = [256] + [512] * ((S - 512) // 512) + [256]
    else:
        xcols = [S]
    xoffs = [sum(xcols[:i]) for i in range(len(xcols))]

    # Shallow-K (Phase A) kernels write fp32 output at a high average rate;
    # deeper out/PSUM buffering rides out transient HBM write backpressure
    # that otherwise backs up through DVE into the PE (measured ~2 µs of
    # mid-pipeline PE gaps). Deep-K (Phase B) is gap-free at 3/3.
    deep = K // 128 >= 16
    with tile.TileContext(nc) as tc:
        with (
            tc.tile_pool(name="xres", bufs=1) as xres,
            tc.tile_pool(name="wpool", bufs=2) as wpool,
            tc.tile_pool(name="opool", bufs=3 if deep else 6) as opool,
            tc.tile_pool(name="ppool", bufs=3 if deep else 4, space="PSUM") as ppool,
        ):
            # Startup bandwidth management: issue W block 0 first, then the
            # small X head chunk; only those two compete for HBM before the
            # first matmul. Later X chunks chain on the previous chunk's
            # DMA; the W block 1 prefetch waits for the mid X chunk.
            wblk0 = wpool.tile([128, KC, NW], BF16, tag="w", name="wblk0")
            nbo0, nbw0 = nbs[0]
            nc.sync.dma_start(out=wblk0[:, :, :nbw0], in_=wt_v[:, :, nbo0:nbo0 + nbw0])

            xch = []
            xdmas = []
            for c, (xo_c, xw_c) in enumerate(zip(xoffs, xcols)):
                xc_t = xres.tile([128, KC, xw_c], BF16, tag=f"x{c}", name=f"x{c}")
                xd = nc.sync.dma_start(
                    out=xc_t[:, :, :], in_=xt_v[:, :, xo_c:xo_c + xw_c]
                )
                if c > 0:
                    add_dep_helper(
                        xd.ins, xdmas[c - 1].ins, sync=True,
                        reason="chain x chunk loads",
                    )
                xch.append(xc_t)
                xdmas.append(xd)

            def x_slice(st):
                pos = st * 128
                for c in range(len(xcols)):
                    if pos < xoffs[c] + xcols[c]:
                        return xch[c], pos - xoffs[c]
                raise AssertionError

            w1_gate = xdmas[min(2, len(xdmas) - 1)]
            for nbi, (nbo, nbw) in enumerate(nbs):
                if nbi == 0:
                    wblk = wblk0
                else:
                    wblk = wpool.tile([128, KC, NW], BF16, tag="w", name=f"wblk{nbi}")
                    wd = nc.sync.dma_start(
                        out=wblk[:, :, :nbw], in_=wt_v[:, :, nbo:nbo + nbw]
                    )
                    if nbi == 1:
                        add_dep_helper(
                            wd.ins, w1_gate.ins, sync=True,
                            reason="hold W prefetch off the startup stream",
                        )
                for st in range(ST):
                    xc, xo = x_slice(st)
                    ps = ppool.tile([128, NW], F32, tag="ps")
                    for kb in range(KC):
                        nc.tensor.matmul(
                            ps[:, :nbw], xc[:, kb, xo:xo + 128], wblk[:, kb, :nbw],
                            start=(kb == 0), stop=(kb == KC - 1),
                        )
                    ot = opool.tile([128, NW], out_dt, tag="o")
                    nc.vector.tensor_copy(out=ot[:, :nbw], in_=ps[:, :nbw])
                    nc.sync.dma_start(
                        out=out[st * 128:(st + 1) * 128, nbo:nbo + nbw],
                        in_=ot[:, :nbw],
                    )
    nc.compile()
    return nc


_KERNEL_CACHE = {}
LAST_EXEC_NS = 0


def _run_mm(key, K, S, N, xts, ws, out_dt=BF16, NW=512):
    global LAST_EXEC_NS
    if key not in _KERNEL_CACHE:
        _KERNEL_CACHE[key] = _build_mm_kernel(K, S, N, out_dt, NW)
    nc = _KERNEL_CACHE[key]
    in_maps = [
        {"xt": np.ascontiguousarray(xts[c]),
         "w": np.ascontiguousarray(ws[c])}
        for c in range(N_CORES)
    ]
    kwargs = {}
    if _TRACE:
        kwargs["trace"] = True
    res = bass_utils.run_bass_kernel_spmd(
        nc, in_maps, core_ids=list(range(N_CORES)), **kwargs
    )
    if res.exec_time_ns:
        LAST_EXEC_NS += res.exec_time_ns
    return res


def kernel(input_ids, enc_W, Wq1, bq1, Wq2, bq2, kb_keys, kb_vals,
           W_ih, b_ih, W_hh, b_hh, W_dec, b_dec):
    input_ids = np.asarray(input_ids)
    enc_W = np.asarray(enc_W, np.float32)
    Wq1 = np.asarray(Wq1, np.float32)
    bq1 = np.asarray(bq1, np.float32)
    Wq2 = np.asarray(Wq2, np.float32)
    bq2 = np.asarray(bq2, np.float32)
    kb_keys = np.asarray(kb_keys, np.float32)
    kb_vals = np.asarray(kb_vals, np.float32)
    W_ih = np.asarray(W_ih, np.float32)
    b_ih = np.asarray(b_ih, np.float32)
    W_hh = np.asarray(W_hh, np.float32)
    b_hh = np.asarray(b_hh, np.float32)
    W_dec = np.asarray(W_dec, np.float32)
    b_dec = np.asarray(b_dec, np.float32)

    # ---- embedding gather (host glue) ----
    emb = enc_W[input_ids]                      # [S, EMB] fp32
    X_T16 = np.ascontiguousarray(emb.T.astype(NP_BF16))   # [EMB, S] bf16

    # ---- Phase A on device: XP = X @ [Wq1_x | W_ih_x^T]; bias added on host
    Wq1_x = Wq1[STATE:, :]                      # [1024, 2048]
    W_ih_xT = np.ascontiguousarray(W_ih[:, :EMB].T)   # [1024, 4096]
    PROJ = np.concatenate([Wq1_x, W_ih_xT], axis=1).astype(NP_BF16)  # [1024, 6144]
    BIAS = np.concatenate([bq1, b_ih + b_hh]).astype(np.float32)     # [6144]
    NSH = 6144 // N_CORES                              # 768
    ws = [PROJ[:, c * NSH:(c + 1) * NSH] for c in range(N_CORES)]
    xts = [X_T16] * N_CORES
    resA = _run_mm("A", EMB, SEQ, NSH, xts, ws, out_dt=F32)
    XP = np.concatenate([resA.results[c]["out"] for c in range(N_CORES)], axis=1)
    XP += BIAS[None, :]
    xq_pre = XP[:, :2048]                        # [S, 2048]  (= x@Wq1_x + bq1)
    xg_pre = XP[:, 2048:]                        # [S, 4096]  (= x@W_ih_x^T + b_ih + b_hh)

    # ---- host sequential scan (glue around device-precomputed projections) ----
    Wq1_h = np.ascontiguousarray(Wq1[:STATE, :])       # [1024, 2048]
    HXW = np.concatenate([Wq1_h, W_hh.T], axis=1)      # [1024, 2048+4096]
    HXW = np.ascontiguousarray(HXW)
    W_ihvT = np.ascontiguousarray(W_ih[:, EMB:].T)     # [512, 4096]
    kb_keys_c = np.ascontiguousarray(kb_keys)
    kb_vals_c = np.ascontiguousarray(kb_vals)
    Wq2_c = np.ascontiguousarray(Wq2)

    hx = np.zeros(STATE, np.float32)
    cx = np.zeros(STATE, np.float32)
    lstm_states = np.empty((SEQ, STATE), np.float32)
    kb_out = np.empty((SEQ, VALUE), np.float32)
    _t0 = time.time()
    for t in range(SEQ):
        if t % 512 == 0:
            print(f"[kernel] scan step {t} ({time.time()-_t0:.1f}s)", flush=True)
        lstm_states[t] = hx
        hp = hx @ HXW                                  # [6144]
        qh = np.tanh(hp[:2048] + xq_pre[t])
        q = qh @ Wq2_c + bq2                           # [256]
        sc = kb_keys_c @ q                             # [NKB]
        sc -= sc.max()
        u = np.exp(sc)
        attn = u / u.sum()
        val = attn @ kb_vals_c                         # [512]
        kb_out[t] = val
        gates = xg_pre[t] + val @ W_ihvT + hp[2048:]   # [4096]
        i_g = gates[:1024]
        f_g = gates[1024:2048]
        g_g = gates[2048:3072]
        o_g = gates[3072:]
        sig_i = 1.0 / (1.0 + np.exp(-i_g))
        sig_f = 1.0 / (1.0 + np.exp(-f_g))
        sig_o = 1.0 / (1.0 + np.exp(-o_g))
        cx = sig_f * cx + sig_i * np.tanh(g_g)
        hx = sig_o * np.tanh(cx)

    # ---- Phase B on device: decoder (host adds bias + log_softmax) ----
    F = np.concatenate([emb, kb_out, lstm_states], axis=1)   # [S, 2560]
    F_T16 = np.ascontiguousarray(F.T.astype(NP_BF16))        # [2560, S] bf16
    VSH = NTOK // N_CORES                                    # 4000
    wdt16 = np.ascontiguousarray(W_dec.T.astype(NP_BF16))    # [2560, 32000]
    ws_b = [np.ascontiguousarray(wdt16[:, c * VSH:(c + 1) * VSH]) for c in range(N_CORES)]
    xts_b = [F_T16] * N_CORES
    resB = _run_mm("B", DEC_IN, SEQ, VSH, xts_b, ws_b, out_dt=BF16)

    logits = np.concatenate(
        [resB.results[c]["out"].astype(np.float32) for c in range(N_CORES)], axis=1
    )
    if os.environ.get("KRNN_DUMP_CACHE"):
        np.savez("/tmp/krnn_f_cache.npz",
                 F_T16=F_T16.view(np.uint16), wdt16=wdt16.view(np.uint16),
                 logits=(F @ W_dec.T.astype(np.float32)).astype(np.float32))
    logits += b_dec[None, :]
    # stable log_softmax on host
    mx = logits.max(axis=1, keepdims=True)
    lse = mx + np.log(np.exp(logits - mx).sum(axis=1, keepdims=True))
    out = logits - lse
    return out.astype(np.float32)


if __name__ == "__main__":
    # smoke test against reference
    sys.path.insert(0, os.path.dirname(os.path.abspath(__file__)))
    import reference
    t0 = time.time()
    inputs = {k: np.asarray(v) for k, v in reference.setup_inputs().items()}
    exp = np.asarray(reference.reference(**inputs))
    t1 = time.time()
    print(f"reference: {t1-t0:.1f}s")
    act = kernel(**inputs)
    t2 = time.time()
    print(f"kernel: {t2-t1:.1f}s")
    err = np.abs(act - exp)
    rel = err.max() / np.abs(exp).max()
    l2 = np.linalg.norm(act - exp) / np.linalg.norm(exp)
    print(f"max abs err {err.max():.3e}  rel(max) {rel:.3e}  rel L2 {l2:.3e}")
